# revision 1
# baseline (speedup 1.0000x reference)
"""Trainium2 Bass kernel for the 25-step spiking MLP (784 -> 1000 -> 10).

Data-parallel over batch: 4096 rows split across 8 NeuronCores (512 each).

Layer-1 state is kept as E = 2*(mem1 - 1), so the LIF step becomes
    E' = beta*E - sgn + cur1E      sgn = Sign(E') in {-1,+1}  (spk=(sgn+1)/2)
with cur1E = 2*(x@W1.T + b1) - 2*(1-beta) - 1 folded into the fc1 weights.
Per step: one DVE scalar_tensor_tensor (beta-decay minus spike), one
tensor_tensor add of the constant cur1E (split DVE/GPSIMD), one ScalarE Sign
producing fp16 +/-1 spikes. The hidden dim is split into 8 independent
column-group chains so the recurrence pipelines across engines, and the sgn
tensor rotates through a buffer pool so early groups run several steps ahead
(hiding the fp32 fc1 prologue).

fc2 consumes the +/-1 fp16 spikes with W2 split into fp16 hi+lo parts
(~fp32 accuracy at 1 cycle/row): rhs weights are 0.5*W2.T and the
always-firing pad unit's weight row carries b2 + 0.5*sum_h W2.T[h].
cur2 lands as [10, 512]; ScalarE copies it to SBUF and TensorE transposes it
to the [128b x (4bt*10o)] layout where layer-2 LIF runs exactly as the
reference; spk2/mem2 records DMA out each step.
"""

import numpy as np

import concourse.bass as bass
import concourse.mybir as mybir
import concourse.tile as tile
from concourse import bacc
from concourse.bass_utils import run_bass_kernel_spmd

F32 = mybir.dt.float32
F16 = mybir.dt.float16
ALU = mybir.AluOpType
ACTF = mybir.ActivationFunctionType

N_CORES = 8
B = 4096
PB = B // N_CORES          # 512 batch rows per core
INP = 784
KA = INP + 1               # ones-row folds the bias in
HID = 1000
HIDP = 1024                # padded hidden (8 x 128)
OUT = 10
T = 25
BETA = 0.95
BIG = 60000.0    # fp16-representable

NHT = HIDP // 128          # 8 hidden-tile column groups (512 cols each)
NBT = PB // 128            # 4 batch partition-tiles
KSPLITS = [(i * 128, min(128, KA - i * 128)) for i in range((KA + 127) // 128)]

POOL_GROUPS = 6
SPLIT_G = (5, 6)       # these groups' adds are half Pool / half DVE
SIGN_PAIR = 2
STT_PAIR = 1
SGN_BUFS = 8               # sgn rotation depth (layer-1 run-ahead)


def _build_program():
    nc = bacc.Bacc("TRN2", target_bir_lowering=False, debug=False,
                   enable_partition_id=False)

    xt_d = nc.dram_tensor("xt", [KA, PB], F32, kind="ExternalInput")
    w1t_d = nc.dram_tensor("w1t", [KA, HIDP], F32, kind="ExternalInput")
    w2h_d = nc.dram_tensor("w2h", [HIDP, OUT], F16, kind="ExternalInput")
    w2l_d = nc.dram_tensor("w2l", [HIDP, OUT], F16, kind="ExternalInput")
    idt_d = nc.dram_tensor("idt", [OUT, OUT], F32, kind="ExternalInput")
    ospk_d = nc.dram_tensor("ospk", [T, PB, OUT], F32, kind="ExternalOutput")
    omem_d = nc.dram_tensor("omem", [T, PB, OUT], F32, kind="ExternalOutput")

    with tile.TileContext(nc) as tc:
        with (
            tc.tile_pool(name="state", bufs=1) as state,
            tc.tile_pool(name="sgnp", bufs=SGN_BUFS) as sgnp,
            tc.tile_pool(name="l2", bufs=3) as l2p,
            tc.tile_pool(name="psum", bufs=2, space="PSUM") as psp,
            tc.tile_pool(name="psum3", bufs=3, space="PSUM") as psp3,
        ):
            # ---- load inputs ----
            xt_t, w1t_t = [], []
            for i, (k0, kk) in enumerate(KSPLITS):
                xk = state.tile([kk, PB], F32, tag=f"xt{i}")
                nc.sync.dma_start(xk[:], xt_d.ap()[k0:k0 + kk, :])
                xt_t.append(xk)
                wk = state.tile([kk, HIDP], F32, tag=f"w1t{i}")
                nc.sync.dma_start(wk[:], w1t_d.ap()[k0:k0 + kk, :])
                w1t_t.append(wk)
            w2h_t = []
            w2l_t = []
            for j in range(NHT):
                wj = state.tile([128, OUT], F16, tag=f"w2h{j}")
                nc.sync.dma_start(wj[:], w2h_d.ap()[j * 128:(j + 1) * 128, :])
                w2h_t.append(wj)
                wl = state.tile([128, OUT], F16, tag=f"w2l{j}")
                nc.sync.dma_start(wl[:], w2l_d.ap()[j * 128:(j + 1) * 128, :])
                w2l_t.append(wl)

            # ---- persistent state [128, NHT*PB]; group g = cols [g*PB,(g+1)*PB) ----
            cur1 = state.tile([128, NHT * PB], F32, tag="cur1")
            est = state.tile([128, NHT * PB], F32, tag="est")

            # 10x10 identity for PE record transposes
            idt = state.tile([OUT, OUT], F32, tag="idt")
            nc.sync.dma_start(idt[:], idt_d.ap())

            # ---- fc1: cur1E (weights pre-scaled on host), exact fp32 ----
            for j in range(NHT):
                ps = psp.tile([128, PB], F32, tag="fc1")
                for i, (k0, kk) in enumerate(KSPLITS):
                    nc.tensor.matmul(
                        ps[:],
                        w1t_t[i][:, j * 128:(j + 1) * 128],
                        xt_t[i][:],
                        start=(i == 0),
                        stop=(i == len(KSPLITS) - 1),
                    )
                half = PB // 2
                nc.scalar.copy(cur1[:, j * PB:j * PB + half], ps[:, :half])
                nc.vector.tensor_copy(cur1[:, j * PB + half:(j + 1) * PB],
                                      ps[:, half:])

            for g0 in range(0, NHT, STT_PAIR):
                lo, hi = g0 * PB, (g0 + STT_PAIR) * PB
                nc.vector.memset(est[:, lo:hi], -2.0)  # E_0 = 2*(mem-1)|mem=0

            sgn_prev = sgnp.tile([128, NHT * PB], F16, tag="sgn")
            for g0 in range(0, NHT, STT_PAIR):
                lo, hi = g0 * PB, (g0 + STT_PAIR) * PB
                nc.vector.memset(sgn_prev[:, lo:hi], -1.0)  # spk_0 = 0

            mem2_prev = l2p.tile([128, NBT * OUT], F32, tag="mem2")
            spk2_prev = l2p.tile([128, NBT * OUT], F32, tag="spk2")
            nc.vector.memset(mem2_prev[:], 0.0)
            nc.vector.memset(spk2_prev[:], 0.0)

            def gs(buf, g):
                return buf[:, g * PB:(g + 1) * PB]

            def l2_block(c2t, t):
                """Layer-2 LIF + records for step t (software-pipelined)."""
                nonlocal mem2_prev, spk2_prev
                c2s = l2p.tile([OUT, PB], F32, tag="c2s")
                nc.scalar.copy(c2s[:], c2t[:])
                c2 = psp.tile([128, NBT * OUT], F32, tag="c2")
                for bt in range(NBT):
                    nc.tensor.transpose(c2[:, bt * OUT:(bt + 1) * OUT],
                                        c2s[:, bt * 128:(bt + 1) * 128],
                                        idt[:])
                mem2 = l2p.tile([128, NBT * OUT], F32, tag="mem2")
                spk2 = l2p.tile([128, NBT * OUT], F32, tag="spk2")
                nc.vector.scalar_tensor_tensor(mem2[:], mem2_prev[:], BETA,
                                               spk2_prev[:], ALU.mult,
                                               ALU.subtract)
                nc.vector.tensor_tensor(mem2[:], mem2[:], c2[:], ALU.add)
                nc.vector.tensor_scalar(spk2[:], mem2[:], 1.0, None, ALU.is_gt)
                base = (t - 1) * PB * OUT
                dims = [[OUT, 128], [128 * OUT, NBT], [1, OUT]]
                nc.sync.dma_start(bass.AP(ospk_d, base, [d[:] for d in dims]),
                                  spk2[:])
                nc.sync.dma_start(bass.AP(omem_d, base, [d[:] for d in dims]),
                                  mem2[:])
                mem2_prev, spk2_prev = mem2, spk2

            pend = []  # (c2t psum tile, step) awaiting layer-2 processing

            # ---- time loop (fully unrolled; groups pipeline across engines) ----
            for t in range(1, T + 1):
                # chain-major emission: each pair runs STT -> adds -> Sign so
                # the four pair-chains stagger across DVE/Pool/ACT instead of
                # convoying phase-by-phase
                sgn = sgnp.tile([128, NHT * PB], F16, tag="sgn")
                for lo_g in range(NHT):
                    hi_g = lo_g + 1
                    lo, hi = lo_g * PB, hi_g * PB
                    # E = beta*E - sgn_{t-1}
                    nc.vector.scalar_tensor_tensor(est[:, lo:hi], est[:, lo:hi],
                                                   BETA, sgn_prev[:, lo:hi],
                                                   ALU.mult, ALU.subtract)
                    for g in range(lo_g, hi_g):
                        # E += cur1E (group SPLIT_G split between Pool and DVE)
                        if g in SPLIT_G:
                            h2 = PB // 2
                            a, b = g * PB, g * PB + h2
                            c = (g + 1) * PB
                            nc.gpsimd.tensor_tensor(est[:, a:b], est[:, a:b],
                                                    cur1[:, a:b], ALU.add)
                            nc.vector.tensor_tensor(est[:, b:c], est[:, b:c],
                                                    cur1[:, b:c], ALU.add)
                        else:
                            eng = nc.gpsimd if g < POOL_GROUPS else nc.vector
                            eng.tensor_tensor(gs(est, g), gs(est, g),
                                              gs(cur1, g), ALU.add)
                for g0 in range(0, 4, SIGN_PAIR):
                    lo, hi = g0 * PB, (g0 + SIGN_PAIR) * PB
                    nc.scalar.activation(sgn[:, lo:hi], est[:, lo:hi],
                                         ACTF.Sign)
                for g0 in (4, 5, 6, 7):
                    lo, hi = g0 * PB, (g0 + 1) * PB
                    nc.scalar.activation(sgn[:, lo:hi], est[:, lo:hi],
                                         ACTF.Sign)

                # layer-2 lagging two steps: transposes/copies have long-ready
                # inputs, so no engine stalls behind fc2_t
                if len(pend) >= 2:
                    l2_block(*pend.pop(0))
                # fc2: cur2T[o, b] = sum_j (w2h_j + w2l_j).T @ sgn_j  (fp16)
                c2t = psp3.tile([OUT, PB], F32, tag="c2t")
                for j in range(NHT):
                    nc.tensor.matmul(c2t[:], w2h_t[j][:], gs(sgn, j),
                                     start=(j == 0), stop=False)
                for j in range(NHT):
                    nc.tensor.matmul(c2t[:], w2l_t[j][:], gs(sgn, j),
                                     start=False, stop=(j == NHT - 1))
                pend.append((c2t, t))
                sgn_prev = sgn
            for p_ in pend:
                l2_block(*p_)

    nc.compile()
    return nc


_NC_CACHE = None


def _prep(x, W1, b1, W2, b2):
    """Host-side input prep shared by all cores."""
    # fc1 produces cur1E = 2*(x@W1.T + b1) - 2*(1-BETA) - 1 directly
    w1t = np.zeros((KA, HIDP), np.float32)
    w1t[:INP, :HID] = 2.0 * W1.T
    w1t[INP, :HID] = 2.0 * b1 - 2.0 * (1.0 - BETA) - 1.0
    w1t[INP, HID] = BIG          # pad unit 1000: sgn=+1 always
    w1t[INP, HID + 1:] = -BIG    # other pad units: sgn=-1 always
    # fc2 on +/-1 spikes: 0.5*W2.T, always-row carries b2 + 0.5*sum(W2.T);
    # fp16 hi + lo split for ~fp32 matmul accuracy at full PE rate
    w2t = np.zeros((HIDP, OUT), np.float32)
    w2t[:HID] = 0.5 * W2.T
    w2t[HID] = b2 + 0.5 * W2.T.sum(axis=0)
    w2h = w2t.astype(np.float16)
    w2l = (w2t - w2h.astype(np.float32)).astype(np.float16)
    xt = np.concatenate([x.T, np.ones((1, x.shape[0]), np.float32)], axis=0)
    return w1t, w2h, w2l, xt


def kernel(x, W1, b1, W2, b2):
    global _NC_CACHE
    x = np.ascontiguousarray(np.asarray(x, np.float32))
    W1 = np.asarray(W1, np.float32)
    b1 = np.asarray(b1, np.float32)
    W2 = np.asarray(W2, np.float32)
    b2 = np.asarray(b2, np.float32)

    w1t, w2h, w2l, xt = _prep(x, W1, b1, W2, b2)

    if _NC_CACHE is None:
        _NC_CACHE = _build_program()
    nc = _NC_CACHE

    in_maps = []
    for c in range(N_CORES):
        sl = slice(c * PB, (c + 1) * PB)
        in_maps.append({
            "xt": np.ascontiguousarray(xt[:, sl]),
            "w1t": w1t,
            "w2h": w2h,
            "w2l": w2l,
            "idt": np.eye(OUT, dtype=np.float32),
        })

    res = run_bass_kernel_spmd(nc, in_maps, core_ids=list(range(N_CORES)))
    kernel.last_results = res

    ospk = np.empty((T, B, OUT), np.float32)
    omem = np.empty((T, B, OUT), np.float32)
    for c in range(N_CORES):
        sl = slice(c * PB, (c + 1) * PB)
        ospk[:, sl, :] = res.results[c]["ospk"]
        omem[:, sl, :] = res.results[c]["omem"]
    return ospk, omem

